# revision 11
# baseline (speedup 1.0000x reference)
"""Trainium2 Bass kernel for nn_CausalSelfAttention (GQA + RoPE + qk-RMSNorm).

Strategy (Megatron-style head parallelism over 8 NeuronCores):
  - Each core owns 2 of the 16 q heads and the matching 1 of 8 kv heads.
  - Per core: QKV projection for its 512 rows of w_attn, RoPE + qk RMS norm,
    causal flash-style attention for its (2 q heads x 2 batches), and a
    partial output projection through its 256 columns of w_proj.
  - Host sums the 8 partial outputs (no on-device collectives).

All tensors are fed to the device pre-swizzled into SBUF-ready
[128, free...] layouts (bf16 for matmul operands).  Matmuls run in bf16 with
fp32 PSUM accumulation; softmax/statistics run in fp32.

Self-contained: hardcodes all shapes from the problem spec.
"""

import math
import numpy as np
import ml_dtypes
from contextlib import ExitStack

# ---- problem constants (hardcoded per spec) ----
B, T, C = 2, 2048, 2048
N_HEAD, N_KV_HEAD, HD = 16, 8, 128
KV_DIM = N_KV_HEAD * HD
EPS = 1.1920929e-07
N_CORES = 8
QH_PER_CORE = N_HEAD // N_CORES          # 2
TOK = B * T                              # 4096
P = 128
TG = 512                                 # token group (matmul N)
NT = TOK // TG                           # 8 token groups
KT = C // P                              # 16 contraction tiles
NGB = T // TG                            # 4 q groups per batch
NJB = T // P                             # 16 k tiles per batch
SCALE = 1.0 / math.sqrt(HD)

BF16 = ml_dtypes.bfloat16

_CACHE = {}


# --------------------------------------------------------------------------
# device program
# --------------------------------------------------------------------------

def _emit(tc, out_ap, t_in):
    import concourse.bass as bass  # noqa: F401
    import concourse.mybir as mybir

    f32 = mybir.dt.float32
    bf16 = mybir.dt.bfloat16
    AF = mybir.ActivationFunctionType
    nc = tc.nc

    x_d = t_in["x_sw"]
    wq_d = t_in["wq_sw"]
    wp_d = t_in["wp_sw"]
    cs_d = t_in["cs_sw"]
    mask_d = t_in["mask_sw"]
    eye_d = t_in["eye_sw"]
    ones_d = t_in["ones_sw"]

    with ExitStack() as root:
        const = root.enter_context(tc.tile_pool(name="const", bufs=1))
        wq_sb = const.tile([P, KT, TG], bf16)
        nc.sync.dma_start(out=wq_sb[:], in_=wq_d)
        wp_sb = const.tile([P, QH_PER_CORE, C], bf16)
        nc.sync.dma_start(out=wp_sb[:], in_=wp_d)
        cs_sb = const.tile([P, 2, T], bf16)
        nc.sync.dma_start(out=cs_sb[:], in_=cs_d)
        mask_sb = const.tile([P, 4, TG], bf16)
        nc.sync.dma_start(out=mask_sb[:], in_=mask_d)
        eye_sb = const.tile([P, P], bf16)
        nc.sync.dma_start(out=eye_sb[:], in_=eye_d)
        ones_sb = const.tile([P, 1], bf16)
        nc.sync.dma_start(out=ones_sb[:], in_=ones_d)
        eps_sb = const.tile([P, 1], f32)
        nc.vector.memset(eps_sb[:], EPS)

        dramp = root.enter_context(tc.tile_pool(name="dscratch", bufs=1, space="DRAM"))
        rn_dram = dramp.tile([3, TOK], f32)
        big = root.enter_context(tc.tile_pool(name="big", bufs=1))
        # post-rope, post-norm q (2 heads) and k, in [d, tok] layout
        qn = [big.tile([P, TOK], bf16, name=f"qn{m}", tag=f"qn{m}") for m in range(3)]
        v_sb = big.tile([P, TOK], bf16, tag="v")
        vT_sb = big.tile([P, 2 * NJB, P], bf16, tag="vT")   # [ktok, (b,j), d]
        yT = [big.tile([P, TOK], bf16, name=f"yT{h}", tag=f"yT{h}") for h in range(QH_PER_CORE)]

        # ---------------- stage 1: QKV projection -------------------------
        with ExitStack() as s1:
            xin = s1.enter_context(tc.tile_pool(name="xin", bufs=2))
            qkv_ps = s1.enter_context(tc.tile_pool(name="qkvps", bufs=3, space="PSUM"))
            ssq_ps = s1.enter_context(tc.tile_pool(name="ssqps", bufs=2, space="PSUM"))
            sqp = s1.enter_context(tc.tile_pool(name="sq", bufs=3))
            srp = s1.enter_context(tc.tile_pool(name="sr", bufs=4))

            for n in range(NT):
                xb = xin.tile([P, KT, TG], bf16)
                nc.sync.dma_start(out=xb[:], in_=x_d[:, n])
                for m in range(4):
                    ps = qkv_ps.tile([P, TG], f32)
                    for k in range(KT):
                        nc.tensor.matmul(
                            ps[:],
                            wq_sb[:, k, m * P:(m + 1) * P],
                            xb[:, k],
                            start=(k == 0),
                            stop=(k == KT - 1),
                        )
                    if m == 3:
                        nc.vector.tensor_copy(v_sb[:, n * TG:(n + 1) * TG], ps[:])
                    else:
                        sq = sqp.tile([P, TG], bf16)
                        nc.scalar.activation(sq[:], ps[:], AF.Square)
                        ssq = ssq_ps.tile([1, TG], f32)
                        nc.tensor.matmul(
                            ssq[:], ones_sb[:], sq[:], start=True, stop=True
                        )
                        nc.vector.tensor_copy(
                            qn[m][:, n * TG:(n + 1) * TG], ps[:]
                        )
                        # rn[m, nslice] = 1/sqrt(ssq/HD + eps) -> DRAM
                        sr = srp.tile([1, TG], f32)
                        nc.scalar.activation(
                            sr[:], ssq[:], AF.Sqrt,
                            bias=eps_sb[0:1, :], scale=1.0 / HD,
                        )
                        nc.vector.reciprocal_approx_fast(sr[:], sr[:])
                        nc.sync.dma_start(
                            out=rn_dram[m:m + 1, n * TG:(n + 1) * TG], in_=sr[:]
                        )


            # ---------------- RoPE + q/k norm ------------------------------
            ropet = s1.enter_context(tc.tile_pool(name="ropet", bufs=2))
            rnbp = s1.enter_context(tc.tile_pool(name="rnb", bufs=2))
            for m in range(3):
                for b in range(B):
                    sl = slice(b * T, (b + 1) * T)
                    t1 = ropet.tile([P, T], bf16, tag="t1")
                    xsw = ropet.tile([P, T], bf16, tag="xsw")
                    # half-swap via sbuf->sbuf DMA (cross-partition move)
                    nc.gpsimd.dma_start(out=xsw[0:64, :], in_=qn[m][64:128, sl])
                    nc.gpsimd.dma_start(out=xsw[64:128, :], in_=qn[m][0:64, sl])
                    nc.vector.tensor_mul(t1[:], qn[m][:, sl], cs_sb[:, 0])
                    # t1 = [x1*c ; x2*c]; xsw*s2n = [x2*s ; -x1*s]
                    nc.vector.tensor_mul(xsw[:], xsw[:], cs_sb[:, 1])
                    nc.vector.tensor_add(qn[m][:, sl], t1[:], xsw[:])
                    # multiply by rms-norm reciprocal (broadcast over partitions)
                    rnb = rnbp.tile([P, T], f32)
                    nc.gpsimd.dma_start(
                        out=rnb[:], in_=rn_dram[m:m + 1, sl].partition_broadcast(P)
                    )
                    nc.vector.tensor_mul(qn[m][:, sl], qn[m][:, sl], rnb[:])

        # ---------------- stage 2: transpose v ----------------------------
        with ExitStack() as s2:
            vt_ps = s2.enter_context(tc.tile_pool(name="vtps", bufs=2, space="PSUM"))
            for blk in range(2 * NJB):
                tp = vt_ps.tile([P, P], bf16)
                nc.tensor.transpose(tp[:], v_sb[:, blk * P:(blk + 1) * P], eye_sb[:])
                nc.vector.tensor_copy(vT_sb[:, blk], tp[:])

        # ---------------- stage 3: attention ------------------------------
        with ExitStack() as s3:
            s_ps = s3.enter_context(tc.tile_pool(name="sps", bufs=3, space="PSUM"))
            y_ps = s3.enter_context(tc.tile_pool(name="yps", bufs=2, space="PSUM"))
            d_ps = s3.enter_context(tc.tile_pool(name="dps", bufs=2, space="PSUM"))
            ptp = s3.enter_context(tc.tile_pool(name="pt", bufs=18))
            denp = s3.enter_context(tc.tile_pool(name="den", bufs=2))
            dendp = s3.enter_context(tc.tile_pool(name="dend", bufs=3, space="DRAM"))
            rdbp = s3.enter_context(tc.tile_pool(name="rdb", bufs=2))

            for b in range(B):
                for qh in range(QH_PER_CORE):
                    q_t, k_t = qn[qh], qn[2]
                    for g in range(NGB):
                        qsl = slice(b * T + g * TG, b * T + (g + 1) * TG)
                        jmax = 4 * g + 3
                        pts = []
                        for j in range(jmax + 1):
                            sp = s_ps.tile([P, TG], f32)
                            nc.tensor.matmul(
                                sp[:],
                                k_t[:, b * T + j * P: b * T + (j + 1) * P],
                                q_t[:, qsl],
                                start=True,
                                stop=True,
                            )
                            pt = ptp.tile([P, TG], bf16)
                            nc.scalar.activation(pt[:], sp[:], AF.Exp, scale=SCALE)
                            if j >= 4 * g:
                                off = j - 4 * g
                                nc.vector.tensor_mul(pt[:], pt[:], mask_sb[:, off])
                            pts.append(pt)
                        yp = y_ps.tile([P, TG], f32)
                        for j, pt in enumerate(pts):
                            nc.tensor.matmul(
                                yp[:], vT_sb[:, b * NJB + j], pt[:],
                                start=(j == 0), stop=(j == jmax),
                            )
                        dp = d_ps.tile([1, TG], f32)
                        for j, pt in enumerate(pts):
                            nc.tensor.matmul(
                                dp[:], ones_sb[:], pt[:],
                                start=(j == 0), stop=(j == jmax),
                            )
                        den = denp.tile([1, TG], f32)
                        nc.vector.reciprocal_approx_fast(den[:], dp[:])
                        dend = dendp.tile([1, TG], f32)
                        nc.gpsimd.dma_start(out=dend[:], in_=den[:])
                        rdb = rdbp.tile([P, TG], f32)
                        nc.gpsimd.dma_start(
                            out=rdb[:], in_=dend[0:1, :].partition_broadcast(P)
                        )
                        nc.vector.tensor_mul(yT[qh][:, qsl], yp[:], rdb[:])

        # ---------------- stage 4: output projection ----------------------
        with ExitStack() as s4:
            o_ps = s4.enter_context(tc.tile_pool(name="ops", bufs=3, space="PSUM"))
            ostgp = s4.enter_context(tc.tile_pool(name="ostg", bufs=2))
            for tt in range(TOK // P):
                ost = ostgp.tile([P, C], f32)
                for og in range(C // TG):
                    op = o_ps.tile([P, TG], f32)
                    nc.tensor.matmul(
                        op[:], yT[0][:, tt * P:(tt + 1) * P],
                        wp_sb[:, 0, og * TG:(og + 1) * TG],
                        start=True, stop=False,
                    )
                    nc.tensor.matmul(
                        op[:], yT[1][:, tt * P:(tt + 1) * P],
                        wp_sb[:, 1, og * TG:(og + 1) * TG],
                        start=False, stop=True,
                    )
                    if og % 2 == 0:
                        nc.vector.tensor_copy(ost[:, og * TG:(og + 1) * TG], op[:])
                    else:
                        nc.scalar.copy(ost[:, og * TG:(og + 1) * TG], op[:])
                nc.sync.dma_start(out=out_ap[tt * P:(tt + 1) * P, :], in_=ost[:])


def build_nc():
    """Build and compile the (single, shared across cores) Bass program."""
    if "nc" in _CACHE:
        return _CACHE["nc"]
    import concourse.mybir as mybir
    import concourse.tile as tile
    from concourse import bacc

    f32 = mybir.dt.float32
    bf16 = mybir.dt.bfloat16

    nc = bacc.Bacc("TRN2", target_bir_lowering=False, debug=False)
    shapes = {
        "x_sw": ((P, NT, KT, TG), bf16),
        "wq_sw": ((P, KT, TG), bf16),
        "wp_sw": ((P, QH_PER_CORE, C), bf16),
        "cs_sw": ((P, 2, T), bf16),
        "mask_sw": ((P, 4, TG), bf16),
        "eye_sw": ((P, P), bf16),
        "ones_sw": ((P, 1), bf16),
    }
    t_in = {
        name: nc.dram_tensor(name, shape, dt, kind="ExternalInput").ap()
        for name, (shape, dt) in shapes.items()
    }
    out_ap = nc.dram_tensor("out", (TOK, C), f32, kind="ExternalOutput").ap()

    with tile.TileContext(nc) as tc:
        _emit(tc, out_ap, t_in)
    nc.compile()
    _CACHE["nc"] = nc
    return nc


# --------------------------------------------------------------------------
# host-side data preparation
# --------------------------------------------------------------------------

def _swizzle_ktiles(a2d):
    """[R*128, F] -> [128, R, F] picking partition-within-tile as leading."""
    r128, f = a2d.shape
    r = r128 // P
    return np.ascontiguousarray(a2d.reshape(r, P, f).transpose(1, 0, 2))


def host_prep(x, w_attn, w_proj, cos, sin):
    x = np.asarray(x, np.float32)
    w_attn = np.asarray(w_attn, np.float32)
    w_proj = np.asarray(w_proj, np.float32)
    cos = np.asarray(cos, np.float32).reshape(T, HD // 2)
    sin = np.asarray(sin, np.float32).reshape(T, HD // 2)

    # x: (B,T,C) -> xT (C, TOK) -> [128, n, k, t]
    xT = x.reshape(TOK, C).T                        # (C, TOK)
    x_sw = (
        xT.reshape(KT, P, NT, TG).transpose(1, 2, 0, 3)  # (P, n, k, t)
    )
    x_sw = np.ascontiguousarray(x_sw).astype(BF16)

    # cos/sin duplicated across both 64-partition halves: [128, 2, T]
    c2 = np.concatenate([cos.T, cos.T], axis=0)     # (128, T)
    s2 = np.concatenate([sin.T, -sin.T], axis=0)    # sign-folded for rope add
    cs_sw = np.stack([c2, s2], axis=1).astype(BF16)  # (128, 2, T)

    # causal masks for the 4 diagonal offsets: keep col >= row + 128*off
    col = np.arange(TG)[None, :]
    row = np.arange(P)[:, None]
    mask_sw = np.stack(
        [(col >= row + P * off) for off in range(4)], axis=1
    ).astype(BF16)                                   # (128, 4, 512)

    eye_sw = np.eye(P, dtype=np.float32).astype(BF16)
    ones_sw = np.ones((P, 1), np.float32).astype(BF16)

    in_maps = []
    for c in range(N_CORES):
        qrows = w_attn[QH_PER_CORE * HD * c: QH_PER_CORE * HD * (c + 1)]
        krows = w_attn[C + HD * c: C + HD * (c + 1)]
        vrows = w_attn[C + KV_DIM + HD * c: C + KV_DIM + HD * (c + 1)]
        w_sel = np.concatenate([qrows, krows, vrows], axis=0)   # (512, C)
        wq_sw = _swizzle_ktiles(w_sel.T).astype(BF16)           # (128, 16, 512)

        wp_sel = w_proj[:, QH_PER_CORE * HD * c: QH_PER_CORE * HD * (c + 1)]
        wp_sw = _swizzle_ktiles(np.ascontiguousarray(wp_sel.T)).astype(BF16)

        in_maps.append({
            "x_sw": x_sw,
            "wq_sw": np.ascontiguousarray(wq_sw.reshape(P, KT, TG)),
            "wp_sw": np.ascontiguousarray(wp_sw.reshape(P, QH_PER_CORE, C)),
            "cs_sw": cs_sw,
            "mask_sw": mask_sw,
            "eye_sw": eye_sw,
            "ones_sw": ones_sw,
        })
    return in_maps


def run_on_hw(in_maps, trace=False, **kwargs):
    from concourse import bass_utils

    nc = build_nc()
    return bass_utils.run_bass_kernel_spmd(
        nc, in_maps, core_ids=list(range(N_CORES)), trace=trace, **kwargs
    )


def kernel(x, w_attn, w_proj, cos, sin):
    in_maps = host_prep(x, w_attn, w_proj, cos, sin)
    res = run_on_hw(in_maps)
    out = np.zeros((TOK, C), np.float64)
    for r in res.results:
        out += r["out"].astype(np.float64)
    return out.astype(np.float32).reshape(B, T, C)


# revision 12
# speedup vs baseline: 1.1672x; 1.1672x over previous
"""Trainium2 Bass kernel for nn_CausalSelfAttention (GQA + RoPE + qk-RMSNorm).

Strategy (Megatron-style head parallelism over 8 NeuronCores):
  - Each core owns 2 of the 16 q heads and the matching 1 of 8 kv heads.
  - Per core: QKV projection for its 512 rows of w_attn, RoPE + qk RMS norm,
    causal flash-style attention for its (2 q heads x 2 batches), and a
    partial output projection through its 256 columns of w_proj.
  - Host sums the 8 partial outputs (no on-device collectives).

All tensors are fed to the device pre-swizzled into SBUF-ready
[128, free...] layouts (bf16 for matmul operands).  Matmuls run in bf16 with
fp32 PSUM accumulation; softmax/statistics run in fp32.

Self-contained: hardcodes all shapes from the problem spec.
"""

import math
import numpy as np
import ml_dtypes
from contextlib import ExitStack

# ---- problem constants (hardcoded per spec) ----
B, T, C = 2, 2048, 2048
N_HEAD, N_KV_HEAD, HD = 16, 8, 128
KV_DIM = N_KV_HEAD * HD
EPS = 1.1920929e-07
N_CORES = 8
QH_PER_CORE = N_HEAD // N_CORES          # 2
TOK = B * T                              # 4096
P = 128
TG = 512                                 # token group (matmul N)
NT = TOK // TG                           # 8 token groups
KT = C // P                              # 16 contraction tiles
NGB = T // TG                            # 4 q groups per batch
NJB = T // P                             # 16 k tiles per batch
SCALE = 1.0 / math.sqrt(HD)

BF16 = ml_dtypes.bfloat16

_CACHE = {}


# --------------------------------------------------------------------------
# device program
# --------------------------------------------------------------------------

def _emit(tc, out_ap, t_in):
    import concourse.bass as bass  # noqa: F401
    import concourse.mybir as mybir

    f32 = mybir.dt.float32
    bf16 = mybir.dt.bfloat16
    AF = mybir.ActivationFunctionType
    nc = tc.nc

    x_d = t_in["x_sw"]
    wq_d = t_in["wq_sw"]
    wp_d = t_in["wp_sw"]
    cs_d = t_in["cs_sw"]
    mask_d = t_in["mask_sw"]
    eye_d = t_in["eye_sw"]
    ones_d = t_in["ones_sw"]

    with ExitStack() as root:
        const = root.enter_context(tc.tile_pool(name="const", bufs=1))
        wq_sb = const.tile([P, KT, TG], bf16)
        for kk in range(4):
            nc.sync.dma_start(out=wq_sb[:, kk * 4:(kk + 1) * 4, :],
                              in_=wq_d[:, kk * 4:(kk + 1) * 4, :])
        wp_sb = const.tile([P, QH_PER_CORE, C], bf16)
        nc.sync.dma_start(out=wp_sb[:], in_=wp_d)
        cs_sb = const.tile([P, 2, T], bf16)
        nc.sync.dma_start(out=cs_sb[:], in_=cs_d)
        mask_sb = const.tile([P, 4, TG], bf16)
        nc.sync.dma_start(out=mask_sb[:], in_=mask_d)
        eye_sb = const.tile([P, P], bf16)
        nc.sync.dma_start(out=eye_sb[:], in_=eye_d)
        ones_sb = const.tile([P, 1], bf16)
        nc.sync.dma_start(out=ones_sb[:], in_=ones_d)
        eps_sb = const.tile([P, 1], f32)
        nc.vector.memset(eps_sb[:], EPS)
        onesm_sb = const.tile([P, P], bf16)
        nc.vector.memset(onesm_sb[:], 1.0)

        big = root.enter_context(tc.tile_pool(name="big", bufs=1))
        # post-rope, post-norm q (2 heads) and k, in [d, tok] layout
        qn = [big.tile([P, TOK], bf16, name=f"qn{m}", tag=f"qn{m}") for m in range(3)]
        v_sb = big.tile([P, TOK], bf16, tag="v")
        vT_sb = big.tile([P, 2 * NJB, P], bf16, tag="vT")   # [ktok, (b,j), d]
        yT = [big.tile([P, TOK], bf16, name=f"yT{h}", tag=f"yT{h}") for h in range(QH_PER_CORE)]

        # ------- stage 1+2: QKV projection + rope/norm + v transpose -------
        with ExitStack() as s1:
            xin = s1.enter_context(tc.tile_pool(name="xin", bufs=2))
            qkv_ps = s1.enter_context(tc.tile_pool(name="qkvps", bufs=3, space="PSUM"))
            ssq_ps = s1.enter_context(tc.tile_pool(name="ssqps", bufs=2, space="PSUM"))
            vt_ps = s1.enter_context(tc.tile_pool(name="vtps", bufs=2, space="PSUM"))
            sqp = s1.enter_context(tc.tile_pool(name="sq", bufs=3))
            srp = s1.enter_context(tc.tile_pool(name="sr", bufs=3))
            ropet = s1.enter_context(tc.tile_pool(name="ropet", bufs=2))

            for b in range(B):
                for nn in range(NT // B):
                    n = b * (NT // B) + nn
                    xb = xin.tile([P, KT, TG], bf16)
                    nc.sync.dma_start(out=xb[:, 0:8, :], in_=x_d[:, n, 0:8, :])
                    nc.sync.dma_start(out=xb[:, 8:16, :], in_=x_d[:, n, 8:16, :])
                    for m in range(4):
                        ps = qkv_ps.tile([P, TG], f32)
                        for k in range(KT):
                            nc.tensor.matmul(
                                ps[:],
                                wq_sb[:, k, m * P:(m + 1) * P],
                                xb[:, k],
                                start=(k == 0),
                                stop=(k == KT - 1),
                            )
                        if m == 3:
                            nc.vector.tensor_copy(v_sb[:, n * TG:(n + 1) * TG], ps[:])
                        else:
                            # rms-norm: broadcast sum-of-squares via all-ones MM
                            sq = sqp.tile([P, TG], bf16)
                            nc.scalar.activation(sq[:], ps[:], AF.Square)
                            ssqb = ssq_ps.tile([P, TG], f32)
                            nc.tensor.matmul(
                                ssqb[:], onesm_sb[:], sq[:], start=True, stop=True
                            )
                            srb = srp.tile([P, TG], f32)
                            nc.scalar.activation(
                                srb[:], ssqb[:], AF.Sqrt,
                                bias=eps_sb[:], scale=1.0 / HD,
                            )
                            nc.vector.reciprocal_approx_fast(srb[:], srb[:])
                            # normalized copy psum -> sbuf (rope comes after;
                            # rotation commutes with the per-token scale)
                            nc.vector.tensor_mul(
                                qn[m][:, n * TG:(n + 1) * TG], ps[:], srb[:]
                            )
                # rope for this batch (k first: attention needs it earliest)
                sl = slice(b * T, (b + 1) * T)
                for m in (2, 0, 1):
                    t1 = ropet.tile([P, T], bf16, tag="t1")
                    xsw = ropet.tile([P, T], bf16, tag="xsw")
                    nc.gpsimd.dma_start(out=xsw[0:64, :], in_=qn[m][64:128, sl])
                    nc.gpsimd.dma_start(out=xsw[64:128, :], in_=qn[m][0:64, sl])
                    nc.vector.tensor_mul(t1[:], qn[m][:, sl], cs_sb[:, 0])
                    # t1 = [x1*c ; x2*c]; xsw*s2n = [x2*s ; -x1*s]
                    nc.vector.tensor_mul(xsw[:], xsw[:], cs_sb[:, 1])
                    nc.vector.tensor_add(qn[m][:, sl], t1[:], xsw[:])
                # v transposes for this batch
                for blk in range(b * NJB, (b + 1) * NJB):
                    tp = vt_ps.tile([P, P], bf16)
                    nc.tensor.transpose(
                        tp[:], v_sb[:, blk * P:(blk + 1) * P], eye_sb[:]
                    )
                    nc.vector.tensor_copy(vT_sb[:, blk], tp[:])

        # ---------------- stage 3: attention ------------------------------
        with ExitStack() as s3:
            s_ps = s3.enter_context(tc.tile_pool(name="sps", bufs=3, space="PSUM"))
            y_ps = s3.enter_context(tc.tile_pool(name="yps", bufs=2, space="PSUM"))
            d_ps = s3.enter_context(tc.tile_pool(name="dps", bufs=2, space="PSUM"))
            ptp = s3.enter_context(tc.tile_pool(name="pt", bufs=18))
            denp = s3.enter_context(tc.tile_pool(name="den", bufs=2))

            for b in range(B):
                for qh in range(QH_PER_CORE):
                    q_t, k_t = qn[qh], qn[2]
                    for g in range(NGB):
                        qsl = slice(b * T + g * TG, b * T + (g + 1) * TG)
                        jmax = 4 * g + 3
                        pts = []
                        for j in range(jmax + 1):
                            sp = s_ps.tile([P, TG], f32)
                            nc.tensor.matmul(
                                sp[:],
                                k_t[:, b * T + j * P: b * T + (j + 1) * P],
                                q_t[:, qsl],
                                start=True,
                                stop=True,
                            )
                            pt = ptp.tile([P, TG], bf16)
                            nc.scalar.activation(pt[:], sp[:], AF.Exp, scale=SCALE)
                            if j >= 4 * g:
                                off = j - 4 * g
                                nc.vector.tensor_mul(pt[:], pt[:], mask_sb[:, off])
                            pts.append(pt)
                        yp = y_ps.tile([P, TG], f32)
                        for j, pt in enumerate(pts):
                            nc.tensor.matmul(
                                yp[:], vT_sb[:, b * NJB + j], pt[:],
                                start=(j == 0), stop=(j == jmax),
                            )
                        # denominator, broadcast to all partitions via ones-mat
                        dp = d_ps.tile([P, TG], f32)
                        for j, pt in enumerate(pts):
                            nc.tensor.matmul(
                                dp[:], onesm_sb[:], pt[:],
                                start=(j == 0), stop=(j == jmax),
                            )
                        den = denp.tile([P, TG], f32)
                        nc.vector.reciprocal_approx_fast(den[:], dp[:])
                        nc.vector.tensor_mul(yT[qh][:, qsl], yp[:], den[:])

        # ---------------- stage 4: output projection ----------------------
        with ExitStack() as s4:
            o_ps = s4.enter_context(tc.tile_pool(name="ops", bufs=4, space="PSUM"))
            ostgp = s4.enter_context(tc.tile_pool(name="ostg", bufs=3))
            for tt in range(TOK // P):
                ost = ostgp.tile([P, C], f32)
                for og in range(C // TG):
                    op = o_ps.tile([P, TG], f32)
                    nc.tensor.matmul(
                        op[:], yT[0][:, tt * P:(tt + 1) * P],
                        wp_sb[:, 0, og * TG:(og + 1) * TG],
                        start=True, stop=False,
                    )
                    nc.tensor.matmul(
                        op[:], yT[1][:, tt * P:(tt + 1) * P],
                        wp_sb[:, 1, og * TG:(og + 1) * TG],
                        start=False, stop=True,
                    )
                    if og % 2 == 0:
                        nc.vector.tensor_copy(ost[:, og * TG:(og + 1) * TG], op[:])
                    else:
                        nc.scalar.copy(ost[:, og * TG:(og + 1) * TG], op[:])
                nc.sync.dma_start(out=out_ap[tt * P:(tt + 1) * P, :], in_=ost[:])


def build_nc():
    """Build and compile the (single, shared across cores) Bass program."""
    if "nc" in _CACHE:
        return _CACHE["nc"]
    import concourse.mybir as mybir
    import concourse.tile as tile
    from concourse import bacc

    f32 = mybir.dt.float32
    bf16 = mybir.dt.bfloat16

    nc = bacc.Bacc("TRN2", target_bir_lowering=False, debug=False)
    shapes = {
        "x_sw": ((P, NT, KT, TG), bf16),
        "wq_sw": ((P, KT, TG), bf16),
        "wp_sw": ((P, QH_PER_CORE, C), bf16),
        "cs_sw": ((P, 2, T), bf16),
        "mask_sw": ((P, 4, TG), bf16),
        "eye_sw": ((P, P), bf16),
        "ones_sw": ((P, 1), bf16),
    }
    t_in = {
        name: nc.dram_tensor(name, shape, dt, kind="ExternalInput").ap()
        for name, (shape, dt) in shapes.items()
    }
    out_ap = nc.dram_tensor("out", (TOK, C), f32, kind="ExternalOutput").ap()

    with tile.TileContext(nc) as tc:
        _emit(tc, out_ap, t_in)
    nc.compile()
    _CACHE["nc"] = nc
    return nc


# --------------------------------------------------------------------------
# host-side data preparation
# --------------------------------------------------------------------------

def _swizzle_ktiles(a2d):
    """[R*128, F] -> [128, R, F] picking partition-within-tile as leading."""
    r128, f = a2d.shape
    r = r128 // P
    return np.ascontiguousarray(a2d.reshape(r, P, f).transpose(1, 0, 2))


def host_prep(x, w_attn, w_proj, cos, sin):
    x = np.asarray(x, np.float32)
    w_attn = np.asarray(w_attn, np.float32)
    w_proj = np.asarray(w_proj, np.float32)
    cos = np.asarray(cos, np.float32).reshape(T, HD // 2)
    sin = np.asarray(sin, np.float32).reshape(T, HD // 2)

    # x: (B,T,C) -> xT (C, TOK) -> [128, n, k, t]
    xT = x.reshape(TOK, C).T                        # (C, TOK)
    x_sw = (
        xT.reshape(KT, P, NT, TG).transpose(1, 2, 0, 3)  # (P, n, k, t)
    )
    x_sw = np.ascontiguousarray(x_sw).astype(BF16)

    # cos/sin duplicated across both 64-partition halves: [128, 2, T]
    c2 = np.concatenate([cos.T, cos.T], axis=0)     # (128, T)
    s2 = np.concatenate([sin.T, -sin.T], axis=0)    # sign-folded for rope add
    cs_sw = np.stack([c2, s2], axis=1).astype(BF16)  # (128, 2, T)

    # causal masks for the 4 diagonal offsets: keep col >= row + 128*off
    col = np.arange(TG)[None, :]
    row = np.arange(P)[:, None]
    mask_sw = np.stack(
        [(col >= row + P * off) for off in range(4)], axis=1
    ).astype(BF16)                                   # (128, 4, 512)

    eye_sw = np.eye(P, dtype=np.float32).astype(BF16)
    ones_sw = np.ones((P, 1), np.float32).astype(BF16)

    in_maps = []
    for c in range(N_CORES):
        qrows = w_attn[QH_PER_CORE * HD * c: QH_PER_CORE * HD * (c + 1)]
        krows = w_attn[C + HD * c: C + HD * (c + 1)]
        vrows = w_attn[C + KV_DIM + HD * c: C + KV_DIM + HD * (c + 1)]
        w_sel = np.concatenate([qrows, krows, vrows], axis=0)   # (512, C)
        wq_sw = _swizzle_ktiles(w_sel.T).astype(BF16)           # (128, 16, 512)

        wp_sel = w_proj[:, QH_PER_CORE * HD * c: QH_PER_CORE * HD * (c + 1)]
        wp_sw = _swizzle_ktiles(np.ascontiguousarray(wp_sel.T)).astype(BF16)

        in_maps.append({
            "x_sw": x_sw,
            "wq_sw": np.ascontiguousarray(wq_sw.reshape(P, KT, TG)),
            "wp_sw": np.ascontiguousarray(wp_sw.reshape(P, QH_PER_CORE, C)),
            "cs_sw": cs_sw,
            "mask_sw": mask_sw,
            "eye_sw": eye_sw,
            "ones_sw": ones_sw,
        })
    return in_maps


def run_on_hw(in_maps, trace=False, **kwargs):
    from concourse import bass_utils

    nc = build_nc()
    return bass_utils.run_bass_kernel_spmd(
        nc, in_maps, core_ids=list(range(N_CORES)), trace=trace, **kwargs
    )


def kernel(x, w_attn, w_proj, cos, sin):
    in_maps = host_prep(x, w_attn, w_proj, cos, sin)
    res = run_on_hw(in_maps)
    out = np.zeros((TOK, C), np.float64)
    for r in res.results:
        out += r["out"].astype(np.float64)
    return out.astype(np.float32).reshape(B, T, C)


# revision 14
# speedup vs baseline: 1.3193x; 1.1303x over previous
"""Trainium2 Bass kernel for nn_CausalSelfAttention (GQA + RoPE + qk-RMSNorm).

Strategy (Megatron-style head parallelism over 8 NeuronCores):
  - Each core owns 2 of the 16 q heads and the matching 1 of 8 kv heads.
  - Per core: QKV projection for its 512 rows of w_attn, RoPE + qk RMS norm,
    causal flash-style attention for its (2 q heads x 2 batches), and a
    partial output projection through its 256 columns of w_proj.
  - Host sums the 8 partial outputs (no on-device collectives).

All tensors are fed to the device pre-swizzled into SBUF-ready
[128, free...] layouts (bf16 for matmul operands).  Matmuls run in bf16 with
fp32 PSUM accumulation; softmax/statistics run in fp32.

Self-contained: hardcodes all shapes from the problem spec.
"""

import math
import numpy as np
import ml_dtypes
from contextlib import ExitStack

# ---- problem constants (hardcoded per spec) ----
B, T, C = 2, 2048, 2048
N_HEAD, N_KV_HEAD, HD = 16, 8, 128
KV_DIM = N_KV_HEAD * HD
EPS = 1.1920929e-07
N_CORES = 8
QH_PER_CORE = N_HEAD // N_CORES          # 2
TOK = B * T                              # 4096
P = 128
TG = 512                                 # token group (matmul N)
NT = TOK // TG                           # 8 token groups
KT = C // P                              # 16 contraction tiles
NGB = T // TG                            # 4 q groups per batch
NJB = T // P                             # 16 k tiles per batch
SCALE = 1.0 / math.sqrt(HD)

BF16 = ml_dtypes.bfloat16

_CACHE = {}


# --------------------------------------------------------------------------
# device program
# --------------------------------------------------------------------------

def _emit(tc, out_ap, t_in):
    import concourse.bass as bass  # noqa: F401
    import concourse.mybir as mybir

    f32 = mybir.dt.float32
    bf16 = mybir.dt.bfloat16
    AF = mybir.ActivationFunctionType
    nc = tc.nc

    x_d = t_in["x_sw"]
    wq_d = t_in["wq_sw"]
    wp_d = t_in["wp_sw"]
    cs_d = t_in["cs_sw"]
    mask_d = t_in["mask_sw"]
    eye_d = t_in["eye_sw"]
    ones_d = t_in["ones_sw"]

    with ExitStack() as root:
        const = root.enter_context(tc.tile_pool(name="const", bufs=1))
        # first QKV matmuls need only wq k0-3 + the first x half: issue those
        # DMAs first so PE starts ~10us earlier; bulk consts follow.
        wq_sb = const.tile([P, KT, TG], bf16)
        nc.sync.dma_start(out=wq_sb[:, 0:4, :], in_=wq_d[:, 0:4, :])
        x0_sb = const.tile([P, KT, TG], bf16, tag="x0")
        nc.sync.dma_start(out=x0_sb[:, 0:4, :], in_=x_d[:, 0, 0:4, :])
        nc.sync.dma_start(out=wq_sb[:, 4:16, :], in_=wq_d[:, 4:16, :])
        nc.sync.dma_start(out=x0_sb[:, 4:16, :], in_=x_d[:, 0, 4:16, :])
        ones_sb = const.tile([P, 1], bf16)
        nc.sync.dma_start(out=ones_sb[:], in_=ones_d)
        eye_sb = const.tile([P, P], bf16)
        nc.sync.dma_start(out=eye_sb[:], in_=eye_d)
        cs_sb = const.tile([P, 2, T], bf16)
        nc.sync.dma_start(out=cs_sb[:], in_=cs_d)
        mask_sb = const.tile([P, 4, TG], bf16)
        nc.sync.dma_start(out=mask_sb[:], in_=mask_d)
        wp_sb = const.tile([P, QH_PER_CORE, C], bf16)
        nc.sync.dma_start(out=wp_sb[:], in_=wp_d)
        eps_sb = const.tile([P, 1], f32)
        nc.vector.memset(eps_sb[:], EPS)
        onesm_sb = const.tile([P, P], bf16)
        nc.vector.memset(onesm_sb[:], 1.0)

        big = root.enter_context(tc.tile_pool(name="big", bufs=1))
        # post-rope, post-norm q (2 heads) and k, in [d, tok] layout
        qn = [big.tile([P, TOK], bf16, name=f"qn{m}", tag=f"qn{m}") for m in range(3)]
        v_sb = big.tile([P, TOK], bf16, tag="v")
        vT_sb = big.tile([P, 2 * NJB, P], bf16, tag="vT")   # [ktok, (b,j), d]
        yT = [big.tile([P, TOK], bf16, name=f"yT{h}", tag=f"yT{h}") for h in range(QH_PER_CORE)]

        # ------- stage 1+2: QKV projection + rope/norm + v transpose -------
        with ExitStack() as s1:
            xin = s1.enter_context(tc.tile_pool(name="xin", bufs=2))
            qkv_ps = s1.enter_context(tc.tile_pool(name="qkvps", bufs=3, space="PSUM"))
            ssq_ps = s1.enter_context(tc.tile_pool(name="ssqps", bufs=2, space="PSUM"))
            vt_ps = s1.enter_context(tc.tile_pool(name="vtps", bufs=2, space="PSUM"))
            sqp = s1.enter_context(tc.tile_pool(name="sq", bufs=3))
            srp = s1.enter_context(tc.tile_pool(name="sr", bufs=3))
            ropet = s1.enter_context(tc.tile_pool(name="ropet", bufs=2))

            for b in range(B):
                for nn in range(NT // B):
                    n = b * (NT // B) + nn
                    if n == 0:
                        xb = x0_sb
                    else:
                        xb = xin.tile([P, KT, TG], bf16)
                        nc.sync.dma_start(out=xb[:, 0:8, :], in_=x_d[:, n, 0:8, :])
                        nc.sync.dma_start(out=xb[:, 8:16, :], in_=x_d[:, n, 8:16, :])
                    for m in range(4):
                        ps = qkv_ps.tile([P, TG], f32)
                        for k in range(KT):
                            nc.tensor.matmul(
                                ps[:],
                                wq_sb[:, k, m * P:(m + 1) * P],
                                xb[:, k],
                                start=(k == 0),
                                stop=(k == KT - 1),
                            )
                        if m == 3:
                            nc.vector.tensor_copy(v_sb[:, n * TG:(n + 1) * TG], ps[:])
                        else:
                            # rms-norm: broadcast sum-of-squares via all-ones MM
                            sq = sqp.tile([P, TG], bf16)
                            nc.scalar.activation(sq[:], ps[:], AF.Square)
                            ssqb = ssq_ps.tile([P, TG], f32)
                            nc.tensor.matmul(
                                ssqb[:], onesm_sb[:], sq[:], start=True, stop=True
                            )
                            srb = srp.tile([P, TG], f32)
                            nc.scalar.activation(
                                srb[:], ssqb[:], AF.Sqrt,
                                bias=eps_sb[:], scale=1.0 / HD,
                            )
                            nc.vector.reciprocal_approx_fast(srb[:], srb[:])
                            # normalized copy psum -> sbuf (rope comes after;
                            # rotation commutes with the per-token scale)
                            nc.vector.tensor_mul(
                                qn[m][:, n * TG:(n + 1) * TG], ps[:], srb[:]
                            )
                # rope for this batch (k first: attention needs it earliest)
                sl = slice(b * T, (b + 1) * T)
                for m in (2, 0, 1):
                    t1 = ropet.tile([P, T], bf16, tag="t1")
                    xsw = ropet.tile([P, T], bf16, tag="xsw")
                    nc.gpsimd.dma_start(out=xsw[0:64, :], in_=qn[m][64:128, sl])
                    nc.gpsimd.dma_start(out=xsw[64:128, :], in_=qn[m][0:64, sl])
                    nc.vector.tensor_mul(t1[:], qn[m][:, sl], cs_sb[:, 0])
                    # t1 = [x1*c ; x2*c]; xsw*s2n = [x2*s ; -x1*s]
                    nc.vector.tensor_mul(xsw[:], xsw[:], cs_sb[:, 1])
                    nc.vector.tensor_add(qn[m][:, sl], t1[:], xsw[:])
                # v transposes for this batch
                for blk in range(b * NJB, (b + 1) * NJB):
                    tp = vt_ps.tile([P, P], bf16)
                    nc.tensor.transpose(
                        tp[:], v_sb[:, blk * P:(blk + 1) * P], eye_sb[:]
                    )
                    nc.vector.tensor_copy(vT_sb[:, blk], tp[:])

        # ---------------- stage 3: attention ------------------------------
        with ExitStack() as s3:
            s_ps = s3.enter_context(tc.tile_pool(name="sps", bufs=2, space="PSUM"))
            y_ps = s3.enter_context(tc.tile_pool(name="yps", bufs=2, space="PSUM"))
            d_ps = s3.enter_context(tc.tile_pool(name="dps", bufs=2, space="PSUM"))
            ptp = s3.enter_context(tc.tile_pool(name="pt", bufs=10))
            denp = s3.enter_context(tc.tile_pool(name="den", bufs=2))

            for b in range(B):
                for qh in range(QH_PER_CORE):
                    q_t, k_t = qn[qh], qn[2]
                    for g in range(NGB):
                        qsl = slice(b * T + g * TG, b * T + (g + 1) * TG)
                        jmax = 4 * g + 3
                        pts = []
                        for pr in range((jmax + 1) // 2):
                            sp2 = s_ps.tile([P, 2, TG], f32)
                            for jj in (0, 1):
                                j = 2 * pr + jj
                                off = (j - 4 * g) * P if j >= 4 * g else 0
                                nc.tensor.matmul(
                                    sp2[:, jj, off:],
                                    k_t[:, b * T + j * P: b * T + (j + 1) * P],
                                    q_t[:, qsl][:, off:],
                                    start=True,
                                    stop=True,
                                )
                            pt2 = ptp.tile([P, 2, TG], bf16)
                            if 2 * pr >= 4 * g:
                                # diagonal pair: per-j exp on the written range
                                for jj in (0, 1):
                                    j = 2 * pr + jj
                                    off = (j - 4 * g) * P
                                    nc.scalar.activation(
                                        pt2[:, jj, off:], sp2[:, jj, off:],
                                        AF.Exp, scale=SCALE,
                                    )
                                    if off:
                                        nc.gpsimd.memset(pt2[:, jj, 0:off], 0.0)
                                    nc.vector.tensor_mul(
                                        pt2[:, jj, off:off + P],
                                        pt2[:, jj, off:off + P],
                                        mask_sb[:, 0, 0:P],
                                    )
                                    pts.append(pt2[:, jj, :])
                            else:
                                nc.scalar.activation(pt2[:], sp2[:], AF.Exp,
                                                     scale=SCALE)
                                pts.append(pt2[:, 0, :])
                                pts.append(pt2[:, 1, :])
                        yp = y_ps.tile([P, TG], f32)
                        for j, pt in enumerate(pts):
                            nc.tensor.matmul(
                                yp[:], vT_sb[:, b * NJB + j], pt[:],
                                start=(j == 0), stop=(j == jmax),
                            )
                        # denominator, broadcast to all partitions via ones-mat
                        dp = d_ps.tile([P, TG], f32)
                        for j, pt in enumerate(pts):
                            nc.tensor.matmul(
                                dp[:], onesm_sb[:], pt[:],
                                start=(j == 0), stop=(j == jmax),
                            )
                        den = denp.tile([P, TG], f32)
                        nc.vector.reciprocal_approx_fast(den[:], dp[:])
                        nc.vector.tensor_mul(yT[qh][:, qsl], yp[:], den[:])

        # ---------------- stage 4: output projection ----------------------
        with ExitStack() as s4:
            o_ps = s4.enter_context(tc.tile_pool(name="ops", bufs=4, space="PSUM"))
            ostgp = s4.enter_context(tc.tile_pool(name="ostg", bufs=3))
            for tt in range(TOK // P):
                ost = ostgp.tile([P, C], bf16)
                for og in range(C // TG):
                    op = o_ps.tile([P, TG], f32)
                    nc.tensor.matmul(
                        op[:], yT[0][:, tt * P:(tt + 1) * P],
                        wp_sb[:, 0, og * TG:(og + 1) * TG],
                        start=True, stop=False,
                    )
                    nc.tensor.matmul(
                        op[:], yT[1][:, tt * P:(tt + 1) * P],
                        wp_sb[:, 1, og * TG:(og + 1) * TG],
                        start=False, stop=True,
                    )
                    if og % 2 == 0:
                        nc.vector.tensor_copy(ost[:, og * TG:(og + 1) * TG], op[:])
                    else:
                        nc.scalar.copy(ost[:, og * TG:(og + 1) * TG], op[:])
                nc.sync.dma_start(out=out_ap[tt * P:(tt + 1) * P, :], in_=ost[:])


def build_nc():
    """Build and compile the (single, shared across cores) Bass program."""
    if "nc" in _CACHE:
        return _CACHE["nc"]
    import concourse.mybir as mybir
    import concourse.tile as tile
    from concourse import bacc

    f32 = mybir.dt.float32  # noqa: F841
    bf16 = mybir.dt.bfloat16

    nc = bacc.Bacc("TRN2", target_bir_lowering=False, debug=False)
    shapes = {
        "x_sw": ((P, NT, KT, TG), bf16),
        "wq_sw": ((P, KT, TG), bf16),
        "wp_sw": ((P, QH_PER_CORE, C), bf16),
        "cs_sw": ((P, 2, T), bf16),
        "mask_sw": ((P, 4, TG), bf16),
        "eye_sw": ((P, P), bf16),
        "ones_sw": ((P, 1), bf16),
    }
    t_in = {
        name: nc.dram_tensor(name, shape, dt, kind="ExternalInput").ap()
        for name, (shape, dt) in shapes.items()
    }
    out_ap = nc.dram_tensor("out", (TOK, C), bf16, kind="ExternalOutput").ap()

    with tile.TileContext(nc) as tc:
        _emit(tc, out_ap, t_in)
    nc.compile()
    _CACHE["nc"] = nc
    return nc


# --------------------------------------------------------------------------
# host-side data preparation
# --------------------------------------------------------------------------

def _swizzle_ktiles(a2d):
    """[R*128, F] -> [128, R, F] picking partition-within-tile as leading."""
    r128, f = a2d.shape
    r = r128 // P
    return np.ascontiguousarray(a2d.reshape(r, P, f).transpose(1, 0, 2))


def host_prep(x, w_attn, w_proj, cos, sin):
    x = np.asarray(x, np.float32)
    w_attn = np.asarray(w_attn, np.float32)
    w_proj = np.asarray(w_proj, np.float32)
    cos = np.asarray(cos, np.float32).reshape(T, HD // 2)
    sin = np.asarray(sin, np.float32).reshape(T, HD // 2)

    # x: (B,T,C) -> xT (C, TOK) -> [128, n, k, t]
    xT = x.reshape(TOK, C).T                        # (C, TOK)
    x_sw = (
        xT.reshape(KT, P, NT, TG).transpose(1, 2, 0, 3)  # (P, n, k, t)
    )
    x_sw = np.ascontiguousarray(x_sw).astype(BF16)

    # cos/sin duplicated across both 64-partition halves: [128, 2, T]
    c2 = np.concatenate([cos.T, cos.T], axis=0)     # (128, T)
    s2 = np.concatenate([sin.T, -sin.T], axis=0)    # sign-folded for rope add
    cs_sw = np.stack([c2, s2], axis=1).astype(BF16)  # (128, 2, T)

    # causal masks for the 4 diagonal offsets: keep col >= row + 128*off
    col = np.arange(TG)[None, :]
    row = np.arange(P)[:, None]
    mask_sw = np.stack(
        [(col >= row + P * off) for off in range(4)], axis=1
    ).astype(BF16)                                   # (128, 4, 512)

    eye_sw = np.eye(P, dtype=np.float32).astype(BF16)
    ones_sw = np.ones((P, 1), np.float32).astype(BF16)

    in_maps = []
    for c in range(N_CORES):
        qrows = w_attn[QH_PER_CORE * HD * c: QH_PER_CORE * HD * (c + 1)]
        krows = w_attn[C + HD * c: C + HD * (c + 1)]
        vrows = w_attn[C + KV_DIM + HD * c: C + KV_DIM + HD * (c + 1)]
        w_sel = np.concatenate([qrows, krows, vrows], axis=0)   # (512, C)
        wq_sw = _swizzle_ktiles(w_sel.T).astype(BF16)           # (128, 16, 512)

        wp_sel = w_proj[:, QH_PER_CORE * HD * c: QH_PER_CORE * HD * (c + 1)]
        wp_sw = _swizzle_ktiles(np.ascontiguousarray(wp_sel.T)).astype(BF16)

        in_maps.append({
            "x_sw": x_sw,
            "wq_sw": np.ascontiguousarray(wq_sw.reshape(P, KT, TG)),
            "wp_sw": np.ascontiguousarray(wp_sw.reshape(P, QH_PER_CORE, C)),
            "cs_sw": cs_sw,
            "mask_sw": mask_sw,
            "eye_sw": eye_sw,
            "ones_sw": ones_sw,
        })
    return in_maps


def run_on_hw(in_maps, trace=False, **kwargs):
    from concourse import bass_utils

    nc = build_nc()
    return bass_utils.run_bass_kernel_spmd(
        nc, in_maps, core_ids=list(range(N_CORES)), trace=trace, **kwargs
    )


def kernel(x, w_attn, w_proj, cos, sin):
    in_maps = host_prep(x, w_attn, w_proj, cos, sin)
    res = run_on_hw(in_maps)
    out = np.zeros((TOK, C), np.float64)
    for r in res.results:
        out += r["out"].astype(np.float64)
    return out.astype(np.float32).reshape(B, T, C)
